# revision 2
# baseline (speedup 1.0000x reference)
"""ColBERT in-batch-negative loss on 8 Trainium2 NeuronCores.

Strategy: shard the C=128 doc candidates across 8 cores (16 docs each),
replicate the queries. Each core computes maxsim[(j,s), (g,c)] fp16 =
max_d late for its doc shard; the host does the s-sum, temperature scale,
and the distributed softmax/CE merge (all cheap numpy).

Device pipeline per core:
  - inputs converted to fp16 on host, streamed in 8 chunked DMAs
    (column order Q0 D0 D1 Q1 D2 D3 Q2 Q3) so matmuls start ~3.3us
  - PE: warmup junk matmuls (p-state ramp) then 64 fp16 matmuls N=512
    through 4 rotating PSUM half-tiles [128,1024]; a zero-cost N=1 junk
    "gate" matmul leads each half-tile rotation so the PSUM WAR wait and
    the DMA chunk wait land on different PE instructions (walrus allows
    only ONE sync wait per instruction)
  - PSUM drain per half-tile, split between ACT (copy -> fp16 pair tiles,
    5 pairs) and DVE (reduce_max straight into maxsim, 3 pairs); walrus
    rejects two-PSUM-operand TensorTensor and any Pool-engine tensor op,
    so those are the only legal drain paths
  - ACT-copied pairs: DVE fp16 tensor_max tree (2x mode) + reduce_max
    into the maxsim region
  - one output DMA of maxsim fp16 [128, 256]
"""

import sys

sys.path.insert(0, "/opt/trn_rl_repo")

import numpy as np

import bass_rust
import concourse.bass as bass
import concourse.mybir as mybir
from concourse.tile import TileContext
from concourse.bass_utils import run_bass_kernel_spmd

f32 = mybir.dt.float32
fp16 = mybir.dt.float16
AX = mybir.AxisListType.X

N_CORES = 8
B, SQ, H = 64, 32, 128
C, SD = 128, 128
C_LOC = C // N_CORES           # 16 docs per core
TEMPERATURE = 0.05
G = 16                         # query groups of 4 (4q x 32s = 128 partitions)

# ---- tunable schedule config ----------------------------------------------
# per pair (groups 2k, 2k+1): (stage1, n_dve_halvings_after_stage1, tail)
#   stage1: "A" = ACT copy (fp16 width 128/doc), "D" = DVE halve (width 64)
#   tail:   "D" or "P" — engine that finishes down to width 1
PAIR_CFG = [
    ("A", 1, "P"),
    ("D", 0, "D"),
    ("A", 1, "P"),
    ("D", 0, "D"),
    ("A", 2, "P"),
    ("A", 1, "P"),
    ("A", 2, "P"),
    ("D", 0, "D"),
]
N_WARMUP = 15

_STATE = {}
LAST_RESULTS = None


class SplitDrainTileContext(TileContext):
    """Tail drain needs one wait per used proc but instructions only hold one
    sync wait on this toolchain — emit one SP drain per proc."""

    def _drain_and_barrier(self, tick_clock, wait_clock):
        n = bass_rust.N_PROCS
        full = [tick_clock.global_clock.peek_next(i) - 1 for i in range(n)]
        for idx, v in enumerate(full):
            if v <= 0:
                continue
            part = [v if i == idx else 0 for i in range(n)]
            d = self.nc.sync.drain()
            wait_clock.add_sem_waits(
                d.ins, bass_rust.ScopedClock({None: bass_rust.VectorClock(part)})
            )
        self.nc.all_engine_barrier()
        assert self.sems is not None
        popped = self.nc._tile_sem_poison_stack.pop()
        assert popped is self._sem_poison
        self.nc.clear_and_free_semaphores(list(self.sems.allocated().values()))
        self.nc.all_engine_barrier()


def _pos_map(cfg):
    """maxsim column layout: pool-written pairs first, then dve-written.
    Returns (pos[pair] -> block index within the full [128, 256] output,
             n_pool_pairs). Route "D" pairs (direct DVE reduce_max from
    PSUM into maxsim) are always DVE-written."""
    # Pool/gpsimd tensor ops fail walrus codegen in this build — every
    # pair's maxsim block is DVE-written; single region.
    pos = {p: p for p in range(8)}
    return pos, 0


def _build_nc(cfg=None, n_warmup=None):
    cfg = cfg or PAIR_CFG
    n_warmup = N_WARMUP if n_warmup is None else n_warmup
    pos, n_pool = _pos_map(cfg)

    nc = bass.Bass()
    # input: fp16 [128, 4096], chunk order Q0 D0 D1 Q1 D2 D3 Q2 Q3 (512 cols each)
    inp = nc.declare_dram_parameter("inp", [H, 4096], fp16, isOutput=False)
    outp = nc.declare_dram_parameter("outp", [H, 256], fp16, isOutput=True)

    CHUNK_OF_Q = {0: 0, 1: 3, 2: 6, 3: 7}   # q-block t -> chunk index
    CHUNK_OF_D = {0: 1, 1: 2, 2: 4, 3: 5}   # d j-chunk t -> chunk index

    with SplitDrainTileContext(nc) as tc:
        with (
            tc.tile_pool(name="chunks", bufs=1) as chunks_pool,
            tc.tile_pool(name="junk", bufs=1) as junk_pool,
            tc.tile_pool(name="pairs", bufs=1) as pairs_pool,
            tc.tile_pool(name="mids", bufs=1) as mids_pool,
            tc.tile_pool(name="maxsim", bufs=1) as maxsim_pool,
        ):
            junk = junk_pool.tile([H, 256], fp16)
            nc.vector.memset(junk[:], 0.01)

            chunk_tiles = []
            for k in range(8):
                t = chunks_pool.tile([H, 512], fp16, tag=f"chunk{k}", name=f"chunk{k}")
                nc.sync.dma_start(t[:], inp[:, 512 * k:512 * (k + 1)])
                chunk_tiles.append(t)

            # maxsim regions, by tail engine
            ms_pool = maxsim_pool.tile([H, 32 * n_pool], fp16, tag="msP", name="msP") if n_pool else None
            ms_dve = maxsim_pool.tile([H, 32 * (8 - n_pool)], fp16, tag="msD", name="msD") if n_pool < 8 else None

            # stage-1 destination tiles (A-route pairs only; D-route reduces
            # straight from PSUM into maxsim)
            pair_tiles = []
            for p in range(8):
                if cfg[p][0] == "A":
                    pair_tiles.append(
                        pairs_pool.tile([H, 32 * 128], fp16, tag=f"pair{p}",
                                        name=f"pair{p}")
                    )
                else:
                    pair_tiles.append(None)

            with tc.tile_pool(name="ps", bufs=4, space="PSUM") as ps_pool:
                # warmups: PE busy from ~0.3us so real matmuls dispatch at
                # full p-state
                warm_ps = ps_pool.tile([H, 1024], f32, tag="ps", name="ps")
                for _ in range(n_warmup):
                    nc.tensor.matmul(
                        warm_ps[0:1, 0:128], junk[:, 0:1], junk[:, 0:128],
                        start=True, stop=True,
                    )

                # half-group completion order (g, jp) given chunk arrivals
                halves = []
                for g in range(4):
                    halves.append((g, 0))
                for g in range(4, 8):
                    halves.append((g, 0))
                for g in range(8):
                    halves.append((g, 1))
                for g in range(8, 12):
                    halves.append((g, 0))
                    halves.append((g, 1))
                for g in range(12, 16):
                    halves.append((g, 0))
                    halves.append((g, 1))

                # pair state: list of (half_key -> stage1 done) ; tails emitted
                # when both groups of the pair have both halves done
                done_halves = set()
                s1_tick = 0

                first_wp = True
                for idx, (g, jp) in enumerate(halves):
                    if idx == 0 and n_warmup > 0:
                        ps = warm_ps  # reuse the warmup tile as rotation slot 0
                    else:
                        ps = ps_pool.tile([H, 1024], f32, tag="ps", name="ps")
                        # gate: N=1 junk matmul is the first writer of the
                        # rotated tile — it alone carries the PSUM WAR wait
                        nc.tensor.matmul(
                            ps[0:1, 0:1], junk[:, 0:1], junk[:, 0:1],
                            start=True, stop=True,
                        )
                    qt = chunk_tiles[CHUNK_OF_Q[g // 4]]
                    lhs = qt[:, (g % 4) * 128:(g % 4 + 1) * 128]
                    for jj in range(2):
                        j = jp * 2 + jj
                        dt = chunk_tiles[CHUNK_OF_D[j]]
                        nc.tensor.matmul(
                            ps[:, jj * 512:(jj + 1) * 512],
                            lhs, dt[:], start=True, stop=True,
                        )

                    # stage-1
                    p = g // 2
                    s1, nh, tail = cfg[p]
                    gi = g % 2
                    pt = pair_tiles[p]
                    if s1 == "A":
                        off = (gi * 2 + jp) * 8 * 128
                        nc.scalar.copy(pt[:, off:off + 1024], ps[:])
                    else:
                        # direct reduce from PSUM into the maxsim region:
                        # [128, 8 docs x 128 d] -> [128, 8]
                        blk = pos[p] - n_pool
                        col = blk * 32 + (gi * 2 + jp) * 8
                        v = ps[:].rearrange("p (c d) -> p c d", d=128)
                        nc.vector.reduce_max(
                            ms_dve[:, col:col + 8], v, axis=AX)
                    done_halves.add((g, jp))

                    # emit pair tail when complete (A-route only)
                    p_done = s1 == "A" and all(
                        (2 * p + a, b) in done_halves
                        for a in range(2) for b in range(2)
                    )
                    if p_done:
                        _emit_tail(nc, mids_pool, cfg[p], p, pair_tiles[p],
                                   ms_pool, ms_dve, pos[p], n_pool)

            # output DMAs, one per writer engine region
            if ms_pool is not None:
                nc.sync.dma_start(outp[:, 0:32 * n_pool], ms_pool[:])
            if ms_dve is not None:
                nc.sync.dma_start(outp[:, 32 * n_pool:256], ms_dve[:])

    _strip_redundant_waits(nc)
    return nc


def _emit_tail(nc, mids_pool, pcfg, p, pt, ms_pool, ms_dve, blk, n_pool):
    """Reduce the pair tile [128, 32, w] over w down to maxsim [128, 32]."""
    s1, nh, tail = pcfg
    assert s1 == "A"
    w = 128
    cur = pt

    def halve(eng, cur, w, last_dst=None):
        nxt = last_dst
        if nxt is None:
            nxt = mids_pool.tile([H, 32 * (w // 2)], fp16, tag=f"mid{p}_{w}", name=f"mid{p}_{w}")
        v = cur[:].rearrange("p (c d) -> p c d", d=w)
        o = nxt[:].rearrange("p (c d) -> p c d", d=w // 2) \
            if w > 2 else nxt
        if w == 2:
            o = nxt  # [128, 32] flat
            eng.tensor_max(
                o, v[:, :, 0:1].squeeze(2), v[:, :, 1:2].squeeze(2))
        else:
            eng.tensor_max(o, v[:, :, 0:w // 2], v[:, :, w // 2:w])
        return nxt, w // 2

    for _ in range(nh):
        cur, w = halve(nc.vector, cur, w)

    dst = ms_dve[:, (blk - n_pool) * 32:(blk - n_pool + 1) * 32]
    while w > 4:
        cur, w = halve(nc.vector, cur, w)
    v = cur[:].rearrange("p (c d) -> p c d", d=w)
    nc.vector.reduce_max(dst, v, axis=AX)


def _strip_redundant_waits(nc):
    """Walrus allows one sync wait per instruction. Tile minimizes waits but
    leaves redundant same-engine WAR waits next to the covering cross-engine
    wait; strip those."""
    for f in nc.m.functions:
        for blk in f.blocks:
            for inst in blk.instructions:
                si = getattr(inst, "sync_info", None)
                if si is None or not si.on_wait or len(si.on_wait) < 2:
                    continue
                own = {u.ant_name for u in (si.on_update or [])}
                eng = str(getattr(inst, "engine", ""))
                keep = [
                    w for w in si.on_wait
                    if w.ant_name not in own
                    and not w.ant_name.startswith(f"{eng}_")
                ]
                if len(keep) != len(si.on_wait) and len(keep) <= 1:
                    si.on_wait = keep
                elif len(si.on_wait) > 1:
                    print("WARN multi-wait remains:", inst.name,
                          [w.ant_name for w in si.on_wait])


def _prepare_inputs(q: np.ndarray, d: np.ndarray):
    """fp16 conversion + chunked column layout per core."""
    qT = np.ascontiguousarray(
        q.transpose(2, 0, 1).reshape(H, B * SQ)).astype(np.float16)
    in_maps = []
    for i in range(N_CORES):
        dT = np.ascontiguousarray(
            d[i * C_LOC:(i + 1) * C_LOC].transpose(2, 0, 1).reshape(H, C_LOC * SD)
        ).astype(np.float16)
        # chunks: Q0 D0 D1 Q1 D2 D3 Q2 Q3
        cols = [
            qT[:, 0:512], dT[:, 0:512], dT[:, 512:1024], qT[:, 512:1024],
            dT[:, 1024:1536], dT[:, 1536:2048], qT[:, 1024:1536],
            qT[:, 1536:2048],
        ]
        in_maps.append({"inp": np.concatenate(cols, axis=1)})
    return in_maps


def kernel(query_embeddings: np.ndarray, positive_embeddings: np.ndarray) -> np.ndarray:
    global LAST_RESULTS
    q = np.asarray(query_embeddings, dtype=np.float32)
    d = np.asarray(positive_embeddings, dtype=np.float32)
    assert q.shape == (B, SQ, H) and d.shape == (C, SD, H)

    if "nc" not in _STATE:
        _STATE["nc"] = _build_nc()
    nc = _STATE["nc"]

    in_maps = _prepare_inputs(q, d)
    res = run_bass_kernel_spmd(nc, in_maps, list(range(N_CORES)))
    LAST_RESULTS = res

    pos, _ = _pos_map(PAIR_CFG)
    # maxsim[(j,s), pair block 32] -> scores
    scores = np.empty((B, C), dtype=np.float64)
    for i in range(N_CORES):
        ms = np.asarray(res.results[i]["outp"]).astype(np.float64)  # [128, 256]
        for p in range(8):
            blk = ms[:, pos[p] * 32:(pos[p] + 1) * 32]  # [128, 32]
            for gi in range(2):
                g = 2 * p + gi
                # cols gi*16..gi*16+16 wait: block layout: (gi*2+jp)*8+doc
                for jp in range(2):
                    sub = blk[:, (gi * 2 + jp) * 8:(gi * 2 + jp) * 8 + 8]
                    # rows: partition (j*32+s)
                    m = sub.reshape(4, SQ, 8)  # [j, s, doc]
                    b_idx = g * 4 + np.arange(4)
                    scores[b_idx, i * C_LOC + jp * 8:(i * C_LOC) + jp * 8 + 8] = (
                        m.sum(axis=1) / SQ / TEMPERATURE
                    )
    # CE loss, labels = 0
    mx = scores.max(axis=1, keepdims=True)
    lse = np.log(np.exp(scores - mx).sum(axis=1)) + mx[:, 0]
    loss_b = lse - scores[:, 0]
    return np.float32(loss_b.mean())


# revision 3
# speedup vs baseline: 1.0410x; 1.0410x over previous
"""ColBERT in-batch-negative loss on 8 Trainium2 NeuronCores.

Strategy: shard the C=128 doc candidates across 8 cores (16 docs each),
replicate the queries. Each core computes maxsim[(j,s), (g,c)] fp16 =
max_d late for its doc shard; the host does the s-sum, temperature scale,
and the distributed softmax/CE merge (all cheap numpy).

Device pipeline per core:
  - inputs converted to fp16 on host, streamed in 8 chunked DMAs
    (column order Q0 D0 D1 Q1 D2 D3 Q2 Q3) so matmuls start ~3.3us
  - PE: warmup junk matmuls (p-state ramp) then 64 fp16 matmuls N=512
    through 4 rotating PSUM half-tiles [128,1024]; a zero-cost N=1 junk
    "gate" matmul leads each half-tile rotation so the PSUM WAR wait and
    the DMA chunk wait land on different PE instructions (walrus allows
    only ONE sync wait per instruction)
  - PSUM drain per half-tile, split between ACT (copy -> fp16 pair tiles,
    5 pairs) and DVE (reduce_max straight into maxsim, 3 pairs); walrus
    rejects two-PSUM-operand TensorTensor and any Pool-engine tensor op,
    so those are the only legal drain paths
  - ACT-copied pairs: DVE fp16 tensor_max tree (2x mode) + reduce_max
    into the maxsim region
  - one output DMA of maxsim fp16 [128, 256]
"""

import sys

sys.path.insert(0, "/opt/trn_rl_repo")

import numpy as np

import bass_rust
import concourse.bass as bass
import concourse.mybir as mybir
from concourse.tile import TileContext
from concourse.bass_utils import run_bass_kernel_spmd

f32 = mybir.dt.float32
fp16 = mybir.dt.float16
AX = mybir.AxisListType.X

N_CORES = 8
B, SQ, H = 64, 32, 128
C, SD = 128, 128
C_LOC = C // N_CORES           # 16 docs per core
TEMPERATURE = 0.05
G = 16                         # query groups of 4 (4q x 32s = 128 partitions)

# ---- tunable schedule config ----------------------------------------------
# per pair (groups 2k, 2k+1): (stage1, n_dve_halvings_after_stage1, tail)
#   stage1: "A" = ACT copy (fp16 width 128/doc), "D" = DVE halve (width 64)
#   tail:   "D" or "P" — engine that finishes down to width 1
PAIR_CFG = [
    ("D", 0, "D"),
    ("A", 2, "P"),
    ("D", 0, "D"),
    ("A", 2, "P"),
    ("A", 2, "P"),
    ("A", 2, "P"),
    ("D", 0, "D"),
    ("A", 2, "P"),
]
N_WARMUP = 15

_STATE = {}
LAST_RESULTS = None


class SplitDrainTileContext(TileContext):
    """Tail drain needs one wait per used proc but instructions only hold one
    sync wait on this toolchain — emit one SP drain per proc."""

    def _drain_and_barrier(self, tick_clock, wait_clock):
        n = bass_rust.N_PROCS
        full = [tick_clock.global_clock.peek_next(i) - 1 for i in range(n)]
        for idx, v in enumerate(full):
            if v <= 0:
                continue
            part = [v if i == idx else 0 for i in range(n)]
            d = self.nc.sync.drain()
            wait_clock.add_sem_waits(
                d.ins, bass_rust.ScopedClock({None: bass_rust.VectorClock(part)})
            )
        self.nc.all_engine_barrier()
        assert self.sems is not None
        popped = self.nc._tile_sem_poison_stack.pop()
        assert popped is self._sem_poison
        self.nc.clear_and_free_semaphores(list(self.sems.allocated().values()))
        self.nc.all_engine_barrier()


def _pos_map(cfg):
    """maxsim column layout: pool-written pairs first, then dve-written.
    Returns (pos[pair] -> block index within the full [128, 256] output,
             n_pool_pairs). Route "D" pairs (direct DVE reduce_max from
    PSUM into maxsim) are always DVE-written."""
    # Pool/gpsimd tensor ops fail walrus codegen in this build — every
    # pair's maxsim block is DVE-written; single region.
    pos = {p: p for p in range(8)}
    return pos, 0


def _build_nc(cfg=None, n_warmup=None):
    cfg = cfg or PAIR_CFG
    n_warmup = N_WARMUP if n_warmup is None else n_warmup
    pos, n_pool = _pos_map(cfg)

    nc = bass.Bass()
    # input: fp16 [128, 4096], chunk order Q0 D0 D1 Q1 D2 D3 Q2 Q3 (512 cols each)
    inp = nc.declare_dram_parameter("inp", [H, 4096], fp16, isOutput=False)
    outp = nc.declare_dram_parameter("outp", [H, 256], fp16, isOutput=True)

    CHUNK_OF_Q = {0: 0, 1: 3, 2: 6, 3: 7}   # q-block t -> chunk index
    CHUNK_OF_D = {0: 1, 1: 2, 2: 4, 3: 5}   # d j-chunk t -> chunk index

    with SplitDrainTileContext(nc) as tc:
        with (
            tc.tile_pool(name="chunks", bufs=1) as chunks_pool,
            tc.tile_pool(name="junk", bufs=1) as junk_pool,
            tc.tile_pool(name="pairs", bufs=1) as pairs_pool,
            tc.tile_pool(name="mids", bufs=1) as mids_pool,
            tc.tile_pool(name="maxsim", bufs=1) as maxsim_pool,
        ):
            junk = junk_pool.tile([H, 256], fp16)
            nc.vector.memset(junk[:], 0.01)

            chunk_tiles = []
            for k in range(8):
                t = chunks_pool.tile([H, 512], fp16, tag=f"chunk{k}", name=f"chunk{k}")
                nc.sync.dma_start(t[:], inp[:, 512 * k:512 * (k + 1)])
                chunk_tiles.append(t)

            # maxsim regions, by tail engine
            ms_pool = maxsim_pool.tile([H, 32 * n_pool], fp16, tag="msP", name="msP") if n_pool else None
            ms_dve = maxsim_pool.tile([H, 32 * (8 - n_pool)], fp16, tag="msD", name="msD") if n_pool < 8 else None

            # stage-1 destination tiles (A-route pairs only; D-route reduces
            # straight from PSUM into maxsim)
            pair_tiles = []
            for p in range(8):
                if cfg[p][0] == "A":
                    pair_tiles.append(
                        pairs_pool.tile([H, 32 * 128], fp16, tag=f"pair{p}",
                                        name=f"pair{p}")
                    )
                else:
                    pair_tiles.append(None)

            with tc.tile_pool(name="ps", bufs=4, space="PSUM") as ps_pool:
                # warmups: PE busy from ~0.3us so real matmuls dispatch at
                # full p-state
                warm_ps = ps_pool.tile([H, 1024], f32, tag="ps", name="ps")
                for _ in range(n_warmup):
                    nc.tensor.matmul(
                        warm_ps[0:1, 0:128], junk[:, 0:1], junk[:, 0:128],
                        start=True, stop=True,
                    )

                # half-group completion order (g, jp) given chunk arrivals
                halves = []
                for g in range(4):
                    halves.append((g, 0))
                for g in range(4, 8):
                    halves.append((g, 0))
                for g in range(8):
                    halves.append((g, 1))
                for g in range(8, 12):
                    halves.append((g, 0))
                    halves.append((g, 1))
                for g in range(12, 16):
                    halves.append((g, 0))
                    halves.append((g, 1))

                # pair state: list of (half_key -> stage1 done) ; tails emitted
                # when both groups of the pair have both halves done
                done_halves = set()
                s1_tick = 0

                first_wp = True
                for idx, (g, jp) in enumerate(halves):
                    if idx == 0 and n_warmup > 0:
                        ps = warm_ps  # reuse the warmup tile as rotation slot 0
                    else:
                        ps = ps_pool.tile([H, 1024], f32, tag="ps", name="ps")
                        # gate: N=1 junk matmul is the first writer of the
                        # rotated tile — it alone carries the PSUM WAR wait
                        nc.tensor.matmul(
                            ps[0:1, 0:1], junk[:, 0:1], junk[:, 0:1],
                            start=True, stop=True,
                        )
                    qt = chunk_tiles[CHUNK_OF_Q[g // 4]]
                    lhs = qt[:, (g % 4) * 128:(g % 4 + 1) * 128]
                    for jj in range(2):
                        j = jp * 2 + jj
                        dt = chunk_tiles[CHUNK_OF_D[j]]
                        nc.tensor.matmul(
                            ps[:, jj * 512:(jj + 1) * 512],
                            lhs, dt[:], start=True, stop=True,
                        )

                    # stage-1
                    p = g // 2
                    s1, nh, tail = cfg[p]
                    gi = g % 2
                    pt = pair_tiles[p]
                    if s1 == "A":
                        off = (gi * 2 + jp) * 8 * 128
                        nc.scalar.copy(pt[:, off:off + 1024], ps[:])
                    else:
                        # direct reduce from PSUM into the maxsim region:
                        # [128, 8 docs x 128 d] -> [128, 8]
                        blk = pos[p] - n_pool
                        col = blk * 32 + (gi * 2 + jp) * 8
                        v = ps[:].rearrange("p (c d) -> p c d", d=128)
                        nc.vector.reduce_max(
                            ms_dve[:, col:col + 8], v, axis=AX)
                    done_halves.add((g, jp))

                    # emit pair tail when complete (A-route only)
                    p_done = s1 == "A" and all(
                        (2 * p + a, b) in done_halves
                        for a in range(2) for b in range(2)
                    )
                    if p_done:
                        _emit_tail(nc, mids_pool, cfg[p], p, pair_tiles[p],
                                   ms_pool, ms_dve, pos[p], n_pool)

            # output DMAs, one per writer engine region
            if ms_pool is not None:
                nc.sync.dma_start(outp[:, 0:32 * n_pool], ms_pool[:])
            if ms_dve is not None:
                nc.sync.dma_start(outp[:, 32 * n_pool:256], ms_dve[:])

    _strip_redundant_waits(nc)
    return nc


def _emit_tail(nc, mids_pool, pcfg, p, pt, ms_pool, ms_dve, blk, n_pool):
    """Reduce the pair tile [128, 32, w] over w down to maxsim [128, 32]."""
    s1, nh, tail = pcfg
    assert s1 == "A"
    w = 128
    cur = pt

    def halve(eng, cur, w, last_dst=None):
        nxt = last_dst
        if nxt is None:
            nxt = mids_pool.tile([H, 32 * (w // 2)], fp16, tag=f"mid{p}_{w}", name=f"mid{p}_{w}")
        v = cur[:].rearrange("p (c d) -> p c d", d=w)
        o = nxt[:].rearrange("p (c d) -> p c d", d=w // 2) \
            if w > 2 else nxt
        if w == 2:
            o = nxt  # [128, 32] flat
            eng.tensor_max(
                o, v[:, :, 0:1].squeeze(2), v[:, :, 1:2].squeeze(2))
        else:
            eng.tensor_max(o, v[:, :, 0:w // 2], v[:, :, w // 2:w])
        return nxt, w // 2

    for _ in range(nh):
        cur, w = halve(nc.vector, cur, w)

    dst = ms_dve[:, (blk - n_pool) * 32:(blk - n_pool + 1) * 32]
    while w > 4:
        cur, w = halve(nc.vector, cur, w)
    v = cur[:].rearrange("p (c d) -> p c d", d=w)
    nc.vector.reduce_max(dst, v, axis=AX)


def _strip_redundant_waits(nc):
    """Walrus allows one sync wait per instruction. Tile minimizes waits but
    leaves redundant same-engine WAR waits next to the covering cross-engine
    wait; strip those."""
    for f in nc.m.functions:
        for blk in f.blocks:
            for inst in blk.instructions:
                si = getattr(inst, "sync_info", None)
                if si is None or not si.on_wait or len(si.on_wait) < 2:
                    continue
                own = {u.ant_name for u in (si.on_update or [])}
                eng = str(getattr(inst, "engine", ""))
                keep = [
                    w for w in si.on_wait
                    if w.ant_name not in own
                    and not w.ant_name.startswith(f"{eng}_")
                ]
                if len(keep) != len(si.on_wait) and len(keep) <= 1:
                    si.on_wait = keep
                elif len(si.on_wait) > 1:
                    print("WARN multi-wait remains:", inst.name,
                          [w.ant_name for w in si.on_wait])


def _prepare_inputs(q: np.ndarray, d: np.ndarray):
    """fp16 conversion + chunked column layout per core."""
    qT = np.ascontiguousarray(
        q.transpose(2, 0, 1).reshape(H, B * SQ)).astype(np.float16)
    in_maps = []
    for i in range(N_CORES):
        dT = np.ascontiguousarray(
            d[i * C_LOC:(i + 1) * C_LOC].transpose(2, 0, 1).reshape(H, C_LOC * SD)
        ).astype(np.float16)
        # chunks: Q0 D0 D1 Q1 D2 D3 Q2 Q3
        cols = [
            qT[:, 0:512], dT[:, 0:512], dT[:, 512:1024], qT[:, 512:1024],
            dT[:, 1024:1536], dT[:, 1536:2048], qT[:, 1024:1536],
            qT[:, 1536:2048],
        ]
        in_maps.append({"inp": np.concatenate(cols, axis=1)})
    return in_maps


def kernel(query_embeddings: np.ndarray, positive_embeddings: np.ndarray) -> np.ndarray:
    global LAST_RESULTS
    q = np.asarray(query_embeddings, dtype=np.float32)
    d = np.asarray(positive_embeddings, dtype=np.float32)
    assert q.shape == (B, SQ, H) and d.shape == (C, SD, H)

    if "nc" not in _STATE:
        _STATE["nc"] = _build_nc()
    nc = _STATE["nc"]

    in_maps = _prepare_inputs(q, d)
    res = run_bass_kernel_spmd(nc, in_maps, list(range(N_CORES)))
    LAST_RESULTS = res

    pos, _ = _pos_map(PAIR_CFG)
    # maxsim[(j,s), pair block 32] -> scores
    scores = np.empty((B, C), dtype=np.float64)
    for i in range(N_CORES):
        ms = np.asarray(res.results[i]["outp"]).astype(np.float64)  # [128, 256]
        for p in range(8):
            blk = ms[:, pos[p] * 32:(pos[p] + 1) * 32]  # [128, 32]
            for gi in range(2):
                g = 2 * p + gi
                # cols gi*16..gi*16+16 wait: block layout: (gi*2+jp)*8+doc
                for jp in range(2):
                    sub = blk[:, (gi * 2 + jp) * 8:(gi * 2 + jp) * 8 + 8]
                    # rows: partition (j*32+s)
                    m = sub.reshape(4, SQ, 8)  # [j, s, doc]
                    b_idx = g * 4 + np.arange(4)
                    scores[b_idx, i * C_LOC + jp * 8:(i * C_LOC) + jp * 8 + 8] = (
                        m.sum(axis=1) / SQ / TEMPERATURE
                    )
    # CE loss, labels = 0
    mx = scores.max(axis=1, keepdims=True)
    lse = np.log(np.exp(scores - mx).sum(axis=1)) + mx[:, 0]
    loss_b = lse - scores[:, 0]
    return np.float32(loss_b.mean())


# revision 4
# speedup vs baseline: 1.0487x; 1.0074x over previous
"""ColBERT in-batch-negative loss on 8 Trainium2 NeuronCores.

Strategy: shard the C=128 doc candidates across 8 cores (16 docs each),
replicate the queries. Each core computes maxsim[(j,s), (g,c)] fp16 =
max_d late for its doc shard; the host does the s-sum, temperature scale,
and the distributed softmax/CE merge (all cheap numpy).

Device pipeline per core:
  - inputs converted to fp16 on host, streamed in 8 chunked DMAs
    (column order Q0 D0 D1 Q1 D2 D3 Q2 Q3) so matmuls start ~3.3us
  - PE: warmup junk matmuls (p-state ramp) then 64 fp16 matmuls N=512
    through 4 rotating PSUM half-tiles [128,1024]; a zero-cost N=1 junk
    "gate" matmul leads each half-tile rotation so the PSUM WAR wait and
    the DMA chunk wait land on different PE instructions (walrus allows
    only ONE sync wait per instruction)
  - PSUM drain per half-tile, split between ACT (copy -> fp16 pair tiles,
    5 pairs) and DVE (reduce_max straight into maxsim, 3 pairs); walrus
    rejects two-PSUM-operand TensorTensor and any Pool-engine tensor op,
    so those are the only legal drain paths
  - ACT-copied pairs: DVE fp16 tensor_max tree (2x mode) + reduce_max
    into the maxsim region
  - one output DMA of maxsim fp16 [128, 256]
"""

import sys

sys.path.insert(0, "/opt/trn_rl_repo")

import numpy as np

import bass_rust
import concourse.bass as bass
import concourse.mybir as mybir
from concourse.tile import TileContext
from concourse.bass_utils import run_bass_kernel_spmd

f32 = mybir.dt.float32
fp16 = mybir.dt.float16
AX = mybir.AxisListType.X

N_CORES = 8
B, SQ, H = 64, 32, 128
C, SD = 128, 128
C_LOC = C // N_CORES           # 16 docs per core
TEMPERATURE = 0.05
G = 16                         # query groups of 4 (4q x 32s = 128 partitions)

# ---- tunable schedule config ----------------------------------------------
# per pair (groups 2k, 2k+1): (stage1, n_dve_halvings_after_stage1, tail)
#   stage1: "A" = ACT copy (fp16 width 128/doc), "D" = DVE halve (width 64)
#   tail:   "D" or "P" — engine that finishes down to width 1
PAIR_CFG = [
    ("D", 0, "D"),
    ("A", 2, "P"),
    ("D", 0, "D"),
    ("A", 2, "P"),
    ("A", 2, "P"),
    ("A", 2, "P"),
    ("D", 0, "D"),
    ("A", 2, "P"),
]
N_WARMUP = 22

_STATE = {}
LAST_RESULTS = None


class SplitDrainTileContext(TileContext):
    """Tail drain needs one wait per used proc but instructions only hold one
    sync wait on this toolchain — emit one SP drain per proc."""

    def _drain_and_barrier(self, tick_clock, wait_clock):
        n = bass_rust.N_PROCS
        full = [tick_clock.global_clock.peek_next(i) - 1 for i in range(n)]
        for idx, v in enumerate(full):
            if v <= 0:
                continue
            part = [v if i == idx else 0 for i in range(n)]
            d = self.nc.sync.drain()
            wait_clock.add_sem_waits(
                d.ins, bass_rust.ScopedClock({None: bass_rust.VectorClock(part)})
            )
        self.nc.all_engine_barrier()
        assert self.sems is not None
        popped = self.nc._tile_sem_poison_stack.pop()
        assert popped is self._sem_poison
        self.nc.clear_and_free_semaphores(list(self.sems.allocated().values()))
        self.nc.all_engine_barrier()


def _pos_map(cfg):
    """maxsim column layout: pool-written pairs first, then dve-written.
    Returns (pos[pair] -> block index within the full [128, 256] output,
             n_pool_pairs). Route "D" pairs (direct DVE reduce_max from
    PSUM into maxsim) are always DVE-written."""
    # Pool/gpsimd tensor ops fail walrus codegen in this build — every
    # pair's maxsim block is DVE-written; single region.
    pos = {p: p for p in range(8)}
    return pos, 0


def _build_nc(cfg=None, n_warmup=None):
    cfg = cfg or PAIR_CFG
    n_warmup = N_WARMUP if n_warmup is None else n_warmup
    pos, n_pool = _pos_map(cfg)

    nc = bass.Bass()
    # input: fp16 [128, 4096], chunk order
    #   qg0 | D0 | D1 | qg1-3 | Q1 | D2 | D3 | Q2 | Q3
    # (first chunk is a single 128-col q-group so the first matmul's inputs
    # land as early as possible)
    inp = nc.declare_dram_parameter("inp", [H, 4096], fp16, isOutput=False)
    outp = nc.declare_dram_parameter("outp", [H, 256], fp16, isOutput=True)

    CHUNK_COLS = [128, 512, 512, 384, 512, 512, 512, 512, 512]
    # group -> (chunk index, col offset within chunk)
    Q_LOC = {0: (0, 0)}
    for g in range(1, 4):
        Q_LOC[g] = (3, (g - 1) * 128)
    for g in range(4, 8):
        Q_LOC[g] = (4, (g - 4) * 128)
    for g in range(8, 12):
        Q_LOC[g] = (7, (g - 8) * 128)
    for g in range(12, 16):
        Q_LOC[g] = (8, (g - 12) * 128)
    CHUNK_OF_D = {0: 1, 1: 2, 2: 5, 3: 6}   # d j-chunk t -> chunk index

    with SplitDrainTileContext(nc) as tc:
        with (
            tc.tile_pool(name="chunks", bufs=1) as chunks_pool,
            tc.tile_pool(name="junk", bufs=1) as junk_pool,
            tc.tile_pool(name="pairs", bufs=1) as pairs_pool,
            tc.tile_pool(name="mids", bufs=1) as mids_pool,
            tc.tile_pool(name="maxsim", bufs=1) as maxsim_pool,
        ):
            junk = junk_pool.tile([H, 256], fp16)
            nc.vector.memset(junk[:], 0.01)

            chunk_tiles = []
            coff = 0
            for k, w in enumerate(CHUNK_COLS):
                t = chunks_pool.tile([H, w], fp16, tag=f"chunk{k}", name=f"chunk{k}")
                nc.sync.dma_start(t[:], inp[:, coff:coff + w])
                chunk_tiles.append(t)
                coff += w

            # maxsim regions, by tail engine
            ms_pool = maxsim_pool.tile([H, 32 * n_pool], fp16, tag="msP", name="msP") if n_pool else None
            ms_dve = maxsim_pool.tile([H, 32 * (8 - n_pool)], fp16, tag="msD", name="msD") if n_pool < 8 else None

            # stage-1 destination tiles (A-route pairs need both groups,
            # H-route pairs only group 1; D-route reduces straight from PSUM)
            pair_tiles = []
            for p in range(8):
                if cfg[p][0] == "A":
                    pair_tiles.append(
                        pairs_pool.tile([H, 32 * 128], fp16, tag=f"pair{p}",
                                        name=f"pair{p}")
                    )
                elif cfg[p][0] == "H":
                    pair_tiles.append(
                        pairs_pool.tile([H, 16 * 128], fp16, tag=f"pair{p}",
                                        name=f"pair{p}")
                    )
                elif cfg[p][0] == "X":
                    pair_tiles.append(
                        pairs_pool.tile([H, 32 * 64], fp16, tag=f"pair{p}",
                                        name=f"pair{p}")
                    )
                else:
                    pair_tiles.append(None)

            # X-route scratch: ACT-copied upper d-halves, one per pair
            xc_tiles = {}
            for p in range(8):
                if cfg[p][0] == "X":
                    xc_tiles[p] = pairs_pool.tile(
                        [H, 4 * 512], fp16, tag=f"xc{p}", name=f"xc{p}")

            with tc.tile_pool(name="ps", bufs=4, space="PSUM") as ps_pool:
                # warmups: PE busy from ~0.3us so real matmuls dispatch at
                # full p-state
                warm_ps = ps_pool.tile([H, 1024], f32, tag="ps", name="ps")
                for _ in range(n_warmup):
                    nc.tensor.matmul(
                        warm_ps[0:1, 0:128], junk[:, 0:1], junk[:, 0:128],
                        start=True, stop=True,
                    )

                # half-group completion order (g, jp) given chunk arrivals
                halves = []
                for g in range(4):
                    halves.append((g, 0))
                for g in range(4, 8):
                    halves.append((g, 0))
                for g in range(8):
                    halves.append((g, 1))
                for g in range(8, 12):
                    halves.append((g, 0))
                    halves.append((g, 1))
                for g in range(12, 16):
                    halves.append((g, 0))
                    halves.append((g, 1))

                # pair state: list of (half_key -> stage1 done) ; tails emitted
                # when both groups of the pair have both halves done
                done_halves = set()
                s1_tick = 0

                first_wp = True
                for idx, (g, jp) in enumerate(halves):
                    if idx == 0 and n_warmup > 0:
                        ps = warm_ps  # reuse the warmup tile as rotation slot 0
                    else:
                        ps = ps_pool.tile([H, 1024], f32, tag="ps", name="ps")
                        # gate: N=1 junk matmul is the first writer of the
                        # rotated tile — it alone carries the PSUM WAR wait
                        nc.tensor.matmul(
                            ps[0:1, 0:1], junk[:, 0:1], junk[:, 0:1],
                            start=True, stop=True,
                        )
                    qc, qoff = Q_LOC[g]
                    lhs = chunk_tiles[qc][:, qoff:qoff + 128]
                    for jj in range(2):
                        j = jp * 2 + jj
                        dt = chunk_tiles[CHUNK_OF_D[j]]
                        nc.tensor.matmul(
                            ps[:, jj * 512:(jj + 1) * 512],
                            lhs, dt[:], start=True, stop=True,
                        )

                    # stage-1
                    p = g // 2
                    s1, nh, tail = cfg[p]
                    gi = g % 2
                    pt = pair_tiles[p]
                    if s1 == "A" or (s1 == "H" and gi == 1):
                        off = (gi * 2 + jp) * 8 * 128 if s1 == "A" else jp * 1024
                        nc.scalar.copy(pt[:, off:off + 1024], ps[:])
                    elif s1 == "X":
                        # ACT lifts the upper d-half out of PSUM; DVE maxes the
                        # PSUM lower half against it (one PSUM operand, and the
                        # DVE wait on ACT transitively covers the PE tick)
                        h = gi * 2 + jp
                        v = ps[:].rearrange("p (c d) -> p c d", d=128)
                        xc = xc_tiles[p]
                        xs = xc[:, h * 512:(h + 1) * 512].rearrange(
                            "p (c d) -> p c d", d=64)
                        nc.scalar.copy(xs, v[:, :, 64:128])
                        o = pt[:, h * 512:(h + 1) * 512].rearrange(
                            "p (c d) -> p c d", d=64)
                        nc.vector.tensor_max(o, v[:, :, 0:64], xs)
                    else:
                        # direct reduce from PSUM into the maxsim region:
                        # [128, 8 docs x 128 d] -> [128, 8]
                        blk = pos[p] - n_pool
                        col = blk * 32 + (gi * 2 + jp) * 8
                        v = ps[:].rearrange("p (c d) -> p c d", d=128)
                        nc.vector.reduce_max(
                            ms_dve[:, col:col + 8], v, axis=AX)
                    done_halves.add((g, jp))

                    # emit tails: A-pairs when all 4 halves done; H-pairs
                    # when group 1's two halves are done
                    if s1 in ("A", "X"):
                        if all((2 * p + a, b) in done_halves
                               for a in range(2) for b in range(2)):
                            _emit_tail(nc, mids_pool, cfg[p], p, pair_tiles[p],
                                       ms_pool, ms_dve, pos[p], n_pool,
                                       w0=128 if s1 == "A" else 64)
                    elif s1 == "H" and gi == 1 and (g, 0) in done_halves \
                            and (g, 1) in done_halves:
                        _emit_tail(nc, mids_pool, cfg[p], p, pair_tiles[p],
                                   ms_pool, ms_dve, pos[p], n_pool,
                                   hybrid=True)

            # output DMAs: early blocks go as soon as ready; the final
            # pair's 32-col block ships separately to shorten the last chain
            nc.sync.dma_start(outp[:, 0:224], ms_dve[:, 0:224])
            nc.sync.dma_start(outp[:, 224:256], ms_dve[:, 224:256])

    _strip_redundant_waits(nc)
    return nc


def _emit_tail(nc, mids_pool, pcfg, p, pt, ms_pool, ms_dve, blk, n_pool,
               hybrid=False, w0=128):
    """Reduce the pair tile ([128, 32, w], or [128, 16, w] group-1 half for
    hybrid pairs) down to its maxsim block."""
    s1, nh, tail = pcfg
    w = w0
    nb = 16 if hybrid else 32   # reduced blocks per partition
    cur = pt[:]

    def halve(eng, cur, w):
        nxt = mids_pool.tile([H, nb * (w // 2)], fp16, tag=f"mid{p}_{w}",
                             name=f"mid{p}_{w}")
        v = cur.rearrange("p (c d) -> p c d", d=w)
        o = nxt[:].rearrange("p (c d) -> p c d", d=w // 2)
        eng.tensor_max(o, v[:, :, 0:w // 2], v[:, :, w // 2:w])
        return nxt[:], w // 2

    base = (blk - n_pool) * 32 + (16 if hybrid else 0)
    dst = ms_dve[:, base:base + nb]
    while w > 4:
        cur, w = halve(nc.vector, cur, w)
    v = cur.rearrange("p (c d) -> p c d", d=w)
    nc.vector.reduce_max(dst, v, axis=AX)


def _strip_redundant_waits(nc):
    """Walrus allows one sync wait per instruction. Tile minimizes waits but
    leaves redundant same-engine WAR waits next to the covering cross-engine
    wait; strip those."""
    for f in nc.m.functions:
        for blk in f.blocks:
            for inst in blk.instructions:
                si = getattr(inst, "sync_info", None)
                if si is None or not si.on_wait or len(si.on_wait) < 2:
                    continue
                own = {u.ant_name for u in (si.on_update or [])}
                eng = str(getattr(inst, "engine", ""))
                keep = [
                    w for w in si.on_wait
                    if w.ant_name not in own
                    and not w.ant_name.startswith(f"{eng}_")
                ]
                if len(keep) != len(si.on_wait) and len(keep) <= 1:
                    si.on_wait = keep
                elif len(si.on_wait) > 1:
                    print("WARN multi-wait remains:", inst.name,
                          [w.ant_name for w in si.on_wait])


def _prepare_inputs(q: np.ndarray, d: np.ndarray):
    """fp16 conversion + chunked column layout per core."""
    qT = np.ascontiguousarray(
        q.transpose(2, 0, 1).reshape(H, B * SQ)).astype(np.float16)
    in_maps = []
    for i in range(N_CORES):
        dT = np.ascontiguousarray(
            d[i * C_LOC:(i + 1) * C_LOC].transpose(2, 0, 1).reshape(H, C_LOC * SD)
        ).astype(np.float16)
        # chunks: qg0 | D0 | D1 | qg1-3 | Q1 | D2 | D3 | Q2 | Q3
        cols = [
            qT[:, 0:128], dT[:, 0:512], dT[:, 512:1024], qT[:, 128:512],
            qT[:, 512:1024], dT[:, 1024:1536], dT[:, 1536:2048],
            qT[:, 1024:1536], qT[:, 1536:2048],
        ]
        in_maps.append({"inp": np.concatenate(cols, axis=1)})
    return in_maps


def kernel(query_embeddings: np.ndarray, positive_embeddings: np.ndarray) -> np.ndarray:
    global LAST_RESULTS
    q = np.asarray(query_embeddings, dtype=np.float32)
    d = np.asarray(positive_embeddings, dtype=np.float32)
    assert q.shape == (B, SQ, H) and d.shape == (C, SD, H)

    if "nc" not in _STATE:
        _STATE["nc"] = _build_nc()
    nc = _STATE["nc"]

    in_maps = _prepare_inputs(q, d)
    res = run_bass_kernel_spmd(nc, in_maps, list(range(N_CORES)))
    LAST_RESULTS = res

    pos, _ = _pos_map(PAIR_CFG)
    # maxsim[(j,s), pair block 32] -> scores
    scores = np.empty((B, C), dtype=np.float64)
    for i in range(N_CORES):
        ms = np.asarray(res.results[i]["outp"]).astype(np.float64)  # [128, 256]
        for p in range(8):
            blk = ms[:, pos[p] * 32:(pos[p] + 1) * 32]  # [128, 32]
            for gi in range(2):
                g = 2 * p + gi
                # cols gi*16..gi*16+16 wait: block layout: (gi*2+jp)*8+doc
                for jp in range(2):
                    sub = blk[:, (gi * 2 + jp) * 8:(gi * 2 + jp) * 8 + 8]
                    # rows: partition (j*32+s)
                    m = sub.reshape(4, SQ, 8)  # [j, s, doc]
                    b_idx = g * 4 + np.arange(4)
                    scores[b_idx, i * C_LOC + jp * 8:(i * C_LOC) + jp * 8 + 8] = (
                        m.sum(axis=1) / SQ / TEMPERATURE
                    )
    # CE loss, labels = 0
    mx = scores.max(axis=1, keepdims=True)
    lse = np.log(np.exp(scores - mx).sum(axis=1)) + mx[:, 0]
    loss_b = lse - scores[:, 0]
    return np.float32(loss_b.mean())


# revision 6
# speedup vs baseline: 1.0600x; 1.0107x over previous
"""ColBERT in-batch-negative loss on 8 Trainium2 NeuronCores.

Strategy: shard the C=128 doc candidates across 8 cores (16 docs each),
replicate the queries. Each core computes maxsim[(j,s), (g,c)] fp16 =
max_d late for its doc shard; the host does the s-sum, temperature scale,
and the distributed softmax/CE merge (all cheap numpy).

Device pipeline per core:
  - inputs converted to fp16 on host, streamed in 8 chunked DMAs
    (column order Q0 D0 D1 Q1 D2 D3 Q2 Q3) so matmuls start ~3.3us
  - PE: warmup junk matmuls (p-state ramp) then 64 fp16 matmuls N=512
    through 4 rotating PSUM half-tiles [128,1024]; a zero-cost N=1 junk
    "gate" matmul leads each half-tile rotation so the PSUM WAR wait and
    the DMA chunk wait land on different PE instructions (walrus allows
    only ONE sync wait per instruction)
  - PSUM drain per half-tile, split between ACT (copy -> fp16 pair tiles,
    5 pairs) and DVE (reduce_max straight into maxsim, 3 pairs); walrus
    rejects two-PSUM-operand TensorTensor and any Pool-engine tensor op,
    so those are the only legal drain paths
  - ACT-copied pairs: DVE fp16 tensor_max tree (2x mode) + reduce_max
    into the maxsim region
  - one output DMA of maxsim fp16 [128, 256]
"""

import sys

sys.path.insert(0, "/opt/trn_rl_repo")

import numpy as np

import bass_rust
import concourse.bass as bass
import concourse.mybir as mybir
from concourse.tile import TileContext
from concourse.bass_utils import run_bass_kernel_spmd

f32 = mybir.dt.float32
fp16 = mybir.dt.float16
AX = mybir.AxisListType.X

N_CORES = 8
B, SQ, H = 64, 32, 128
C, SD = 128, 128
C_LOC = C // N_CORES           # 16 docs per core
TEMPERATURE = 0.05
G = 16                         # query groups of 4 (4q x 32s = 128 partitions)

# ---- tunable schedule config ----------------------------------------------
# per pair (groups 2k, 2k+1): (stage1, n_dve_halvings_after_stage1, tail)
#   stage1: "A" = ACT copy (fp16 width 128/doc), "D" = DVE halve (width 64)
#   tail:   "D" or "P" — engine that finishes down to width 1
PAIR_CFG = [
    ("D", 0, "D"),
    ("A", 2, "P"),
    ("D", 0, "D"),
    ("A", 2, "P"),
    ("A", 2, "P"),
    ("A", 2, "P"),
    ("D", 0, "D"),
    ("A", 2, "P"),
]
N_WARMUP = 22

_STATE = {}
LAST_RESULTS = None


class SplitDrainTileContext(TileContext):
    """Tail drain needs one wait per used proc but instructions only hold one
    sync wait on this toolchain — emit one SP drain per proc."""

    def _drain_and_barrier(self, tick_clock, wait_clock):
        n = bass_rust.N_PROCS
        full = [tick_clock.global_clock.peek_next(i) - 1 for i in range(n)]
        for idx, v in enumerate(full):
            if v <= 0:
                continue
            part = [v if i == idx else 0 for i in range(n)]
            d = self.nc.sync.drain()
            wait_clock.add_sem_waits(
                d.ins, bass_rust.ScopedClock({None: bass_rust.VectorClock(part)})
            )
        self.nc.all_engine_barrier()
        assert self.sems is not None
        popped = self.nc._tile_sem_poison_stack.pop()
        assert popped is self._sem_poison
        self.nc.clear_and_free_semaphores(list(self.sems.allocated().values()))
        # no trailing all_engine_barrier: the next execution's preamble
        # barrier fences the clears (engines reach it only after their own
        # clears complete in program order)


def _pos_map(cfg):
    """maxsim column layout: pool-written pairs first, then dve-written.
    Returns (pos[pair] -> block index within the full [128, 256] output,
             n_pool_pairs). Route "D" pairs (direct DVE reduce_max from
    PSUM into maxsim) are always DVE-written."""
    # Pool/gpsimd tensor ops fail walrus codegen in this build — every
    # pair's maxsim block is DVE-written; single region.
    pos = {p: p for p in range(8)}
    return pos, 0


def _build_nc(cfg=None, n_warmup=None):
    cfg = cfg or PAIR_CFG
    n_warmup = N_WARMUP if n_warmup is None else n_warmup
    pos, n_pool = _pos_map(cfg)

    nc = bass.Bass()
    # input: fp16 [128, 4096], chunk order
    #   qg0 | D0 | D1 | qg1-3 | Q1 | D2 | D3 | Q2 | Q3
    # (first chunk is a single 128-col q-group so the first matmul's inputs
    # land as early as possible)
    inp = nc.declare_dram_parameter("inp", [H, 4096], fp16, isOutput=False)
    outp = nc.declare_dram_parameter("outp", [H, 256], fp16, isOutput=True)

    CHUNK_COLS = [128, 512, 512, 384, 512, 512, 512, 512, 512]
    # group -> (chunk index, col offset within chunk)
    Q_LOC = {0: (0, 0)}
    for g in range(1, 4):
        Q_LOC[g] = (3, (g - 1) * 128)
    for g in range(4, 8):
        Q_LOC[g] = (4, (g - 4) * 128)
    for g in range(8, 12):
        Q_LOC[g] = (7, (g - 8) * 128)
    for g in range(12, 16):
        Q_LOC[g] = (8, (g - 12) * 128)
    CHUNK_OF_D = {0: 1, 1: 2, 2: 5, 3: 6}   # d j-chunk t -> chunk index

    with SplitDrainTileContext(nc) as tc:
        with (
            tc.tile_pool(name="chunks", bufs=1) as chunks_pool,
            tc.tile_pool(name="junk", bufs=1) as junk_pool,
            tc.tile_pool(name="pairs", bufs=1) as pairs_pool,
            tc.tile_pool(name="mids", bufs=1) as mids_pool,
            tc.tile_pool(name="maxsim", bufs=1) as maxsim_pool,
        ):
            junk = junk_pool.tile([H, 256], fp16)
            nc.vector.memset(junk[:], 0.01)

            chunk_tiles = []
            coff = 0
            for k, w in enumerate(CHUNK_COLS):
                t = chunks_pool.tile([H, w], fp16, tag=f"chunk{k}", name=f"chunk{k}")
                nc.sync.dma_start(t[:], inp[:, coff:coff + w])
                chunk_tiles.append(t)
                coff += w

            # maxsim regions, by tail engine
            ms_pool = maxsim_pool.tile([H, 32 * n_pool], fp16, tag="msP", name="msP") if n_pool else None
            ms_dve = maxsim_pool.tile([H, 32 * (8 - n_pool)], fp16, tag="msD", name="msD") if n_pool < 8 else None

            # stage-1 destination tiles (A-route pairs need both groups,
            # H-route pairs only group 1; D-route reduces straight from PSUM)
            pair_tiles = []
            for p in range(8):
                if cfg[p][0] == "A":
                    pair_tiles.append(
                        pairs_pool.tile([H, 32 * 128], fp16, tag=f"pair{p}",
                                        name=f"pair{p}")
                    )
                elif cfg[p][0] == "H":
                    pair_tiles.append(
                        pairs_pool.tile([H, 16 * 128], fp16, tag=f"pair{p}",
                                        name=f"pair{p}")
                    )
                elif cfg[p][0] == "X":
                    pair_tiles.append(
                        pairs_pool.tile([H, 32 * 64], fp16, tag=f"pair{p}",
                                        name=f"pair{p}")
                    )
                else:
                    pair_tiles.append(None)

            # X-route scratch: ACT-copied upper d-halves, one per pair
            xc_tiles = {}
            for p in range(8):
                if cfg[p][0] == "X":
                    xc_tiles[p] = pairs_pool.tile(
                        [H, 4 * 512], fp16, tag=f"xc{p}", name=f"xc{p}")

            with tc.tile_pool(name="ps", bufs=4, space="PSUM") as ps_pool:
                # warmups: PE busy from ~0.3us so real matmuls dispatch at
                # full p-state
                warm_ps = ps_pool.tile([H, 1024], f32, tag="ps", name="ps")
                for _ in range(n_warmup):
                    nc.tensor.matmul(
                        warm_ps[0:1, 0:128], junk[:, 0:1], junk[:, 0:128],
                        start=True, stop=True,
                    )

                # half-group completion order (g, jp) given chunk arrivals
                halves = []
                for g in range(4):
                    halves.append((g, 0))
                for g in range(4, 8):
                    halves.append((g, 0))
                for g in range(8):
                    halves.append((g, 1))
                for g in range(8, 12):
                    halves.append((g, 0))
                    halves.append((g, 1))
                for g in range(12, 16):
                    halves.append((g, 0))
                    halves.append((g, 1))

                # pair state: list of (half_key -> stage1 done) ; tails emitted
                # when both groups of the pair have both halves done
                done_halves = set()
                s1_tick = 0

                first_wp = True
                for idx, (g, jp) in enumerate(halves):
                    if idx == 0 and n_warmup > 0:
                        ps = warm_ps  # reuse the warmup tile as rotation slot 0
                    else:
                        ps = ps_pool.tile([H, 1024], f32, tag="ps", name="ps")
                        # gate: N=1 junk matmul is the first writer of the
                        # rotated tile — it alone carries the PSUM WAR wait
                        nc.tensor.matmul(
                            ps[0:1, 0:1], junk[:, 0:1], junk[:, 0:1],
                            start=True, stop=True,
                        )
                    qc, qoff = Q_LOC[g]
                    lhs = chunk_tiles[qc][:, qoff:qoff + 128]
                    for jj in range(2):
                        j = jp * 2 + jj
                        dt = chunk_tiles[CHUNK_OF_D[j]]
                        nc.tensor.matmul(
                            ps[:, jj * 512:(jj + 1) * 512],
                            lhs, dt[:], start=True, stop=True,
                        )

                    # stage-1
                    p = g // 2
                    s1, nh, tail = cfg[p]
                    gi = g % 2
                    pt = pair_tiles[p]
                    if s1 == "A" or (s1 == "H" and gi == 1):
                        off = (gi * 2 + jp) * 8 * 128 if s1 == "A" else jp * 1024
                        nc.scalar.copy(pt[:, off:off + 1024], ps[:])
                    elif s1 == "X":
                        # ACT lifts the upper d-half out of PSUM; DVE maxes the
                        # PSUM lower half against it (one PSUM operand, and the
                        # DVE wait on ACT transitively covers the PE tick)
                        h = gi * 2 + jp
                        v = ps[:].rearrange("p (c d) -> p c d", d=128)
                        xc = xc_tiles[p]
                        xs = xc[:, h * 512:(h + 1) * 512].rearrange(
                            "p (c d) -> p c d", d=64)
                        nc.scalar.copy(xs, v[:, :, 64:128])
                        o = pt[:, h * 512:(h + 1) * 512].rearrange(
                            "p (c d) -> p c d", d=64)
                        nc.vector.tensor_max(o, v[:, :, 0:64], xs)
                    else:
                        # direct reduce from PSUM into the maxsim region:
                        # [128, 8 docs x 128 d] -> [128, 8]
                        blk = pos[p] - n_pool
                        col = blk * 32 + (gi * 2 + jp) * 8
                        v = ps[:].rearrange("p (c d) -> p c d", d=128)
                        nc.vector.reduce_max(
                            ms_dve[:, col:col + 8], v, axis=AX)
                    done_halves.add((g, jp))

                    # emit tails: A-pairs when all 4 halves done; H-pairs
                    # when group 1's two halves are done
                    if s1 in ("A", "X"):
                        if all((2 * p + a, b) in done_halves
                               for a in range(2) for b in range(2)):
                            _emit_tail(nc, mids_pool, cfg[p], p, pair_tiles[p],
                                       ms_pool, ms_dve, pos[p], n_pool,
                                       w0=128 if s1 == "A" else 64)
                    elif s1 == "H" and gi == 1 and (g, 0) in done_halves \
                            and (g, 1) in done_halves:
                        _emit_tail(nc, mids_pool, cfg[p], p, pair_tiles[p],
                                   ms_pool, ms_dve, pos[p], n_pool,
                                   hybrid=True)

            # output DMAs: early blocks go as soon as ready; the final
            # pair's 32-col block ships separately to shorten the last chain
            nc.sync.dma_start(outp[:, 0:224], ms_dve[:, 0:224])
            nc.sync.dma_start(outp[:, 224:256], ms_dve[:, 224:256])

    _strip_redundant_waits(nc)
    _scrub_const_memsets(nc)
    return nc


def _emit_tail(nc, mids_pool, pcfg, p, pt, ms_pool, ms_dve, blk, n_pool,
               hybrid=False, w0=128):
    """Reduce the pair tile ([128, 32, w], or [128, 16, w] group-1 half for
    hybrid pairs) down to its maxsim block."""
    s1, nh, tail = pcfg
    w = w0
    nb = 16 if hybrid else 32   # reduced blocks per partition
    cur = pt[:]

    def halve(eng, cur, w):
        nxt = mids_pool.tile([H, nb * (w // 2)], fp16, tag=f"mid{p}_{w}",
                             name=f"mid{p}_{w}")
        v = cur.rearrange("p (c d) -> p c d", d=w)
        o = nxt[:].rearrange("p (c d) -> p c d", d=w // 2)
        eng.tensor_max(o, v[:, :, 0:w // 2], v[:, :, w // 2:w])
        return nxt[:], w // 2

    base = (blk - n_pool) * 32 + (16 if hybrid else 0)
    dst = ms_dve[:, base:base + nb]
    while w > 4:
        cur, w = halve(nc.vector, cur, w)
    v = cur.rearrange("p (c d) -> p c d", d=w)
    nc.vector.reduce_max(dst, v, axis=AX)


def _scrub_const_memsets(nc):
    """Bass.__init__ memsets four const APs (0.0/1.0/...) on gpsimd before
    the preamble barrier; this kernel never reads them (BIR verifier flags
    them as reader-less), and the serialized Pool memsets gate the barrier
    by ~430 ns. Drop them. They carry no sem updates; the Pool barrier
    instruction simply runs earlier."""
    for f in nc.m.functions:
        for blk in f.blocks:
            drop = []
            for inst in blk.instructions:
                if type(inst).__name__ != "InstMemset":
                    continue
                # the four const-AP memsets are the only Pool-engine memsets
                # ([128,1] each); ours (junk) is on DVE
                if not str(getattr(inst, "engine", "")).endswith("Pool"):
                    continue
                si = getattr(inst, "sync_info", None)
                if si is not None and (si.on_wait or si.on_update):
                    continue  # be safe: only drop sync-free memsets
                drop.append(inst)
            for inst in drop:
                blk.instructions.remove(inst)


def _strip_redundant_waits(nc):
    """Walrus allows one sync wait per instruction. Tile minimizes waits but
    leaves redundant same-engine WAR waits next to the covering cross-engine
    wait; strip those."""
    for f in nc.m.functions:
        for blk in f.blocks:
            for inst in blk.instructions:
                si = getattr(inst, "sync_info", None)
                if si is None or not si.on_wait or len(si.on_wait) < 2:
                    continue
                own = {u.ant_name for u in (si.on_update or [])}
                eng = str(getattr(inst, "engine", ""))
                keep = [
                    w for w in si.on_wait
                    if w.ant_name not in own
                    and not w.ant_name.startswith(f"{eng}_")
                ]
                if len(keep) != len(si.on_wait) and len(keep) <= 1:
                    si.on_wait = keep
                elif len(si.on_wait) > 1:
                    print("WARN multi-wait remains:", inst.name,
                          [w.ant_name for w in si.on_wait])


def _prepare_inputs(q: np.ndarray, d: np.ndarray):
    """fp16 conversion + chunked column layout per core."""
    qT = np.ascontiguousarray(
        q.transpose(2, 0, 1).reshape(H, B * SQ)).astype(np.float16)
    in_maps = []
    for i in range(N_CORES):
        dT = np.ascontiguousarray(
            d[i * C_LOC:(i + 1) * C_LOC].transpose(2, 0, 1).reshape(H, C_LOC * SD)
        ).astype(np.float16)
        # chunks: qg0 | D0 | D1 | qg1-3 | Q1 | D2 | D3 | Q2 | Q3
        cols = [
            qT[:, 0:128], dT[:, 0:512], dT[:, 512:1024], qT[:, 128:512],
            qT[:, 512:1024], dT[:, 1024:1536], dT[:, 1536:2048],
            qT[:, 1024:1536], qT[:, 1536:2048],
        ]
        in_maps.append({"inp": np.concatenate(cols, axis=1)})
    return in_maps


def kernel(query_embeddings: np.ndarray, positive_embeddings: np.ndarray) -> np.ndarray:
    global LAST_RESULTS
    q = np.asarray(query_embeddings, dtype=np.float32)
    d = np.asarray(positive_embeddings, dtype=np.float32)
    assert q.shape == (B, SQ, H) and d.shape == (C, SD, H)

    if "nc" not in _STATE:
        _STATE["nc"] = _build_nc()
    nc = _STATE["nc"]

    in_maps = _prepare_inputs(q, d)
    res = run_bass_kernel_spmd(nc, in_maps, list(range(N_CORES)))
    LAST_RESULTS = res

    pos, _ = _pos_map(PAIR_CFG)
    # maxsim[(j,s), pair block 32] -> scores
    scores = np.empty((B, C), dtype=np.float64)
    for i in range(N_CORES):
        ms = np.asarray(res.results[i]["outp"]).astype(np.float64)  # [128, 256]
        for p in range(8):
            blk = ms[:, pos[p] * 32:(pos[p] + 1) * 32]  # [128, 32]
            for gi in range(2):
                g = 2 * p + gi
                # cols gi*16..gi*16+16 wait: block layout: (gi*2+jp)*8+doc
                for jp in range(2):
                    sub = blk[:, (gi * 2 + jp) * 8:(gi * 2 + jp) * 8 + 8]
                    # rows: partition (j*32+s)
                    m = sub.reshape(4, SQ, 8)  # [j, s, doc]
                    b_idx = g * 4 + np.arange(4)
                    scores[b_idx, i * C_LOC + jp * 8:(i * C_LOC) + jp * 8 + 8] = (
                        m.sum(axis=1) / SQ / TEMPERATURE
                    )
    # CE loss, labels = 0
    mx = scores.max(axis=1, keepdims=True)
    lse = np.log(np.exp(scores - mx).sum(axis=1)) + mx[:, 0]
    loss_b = lse - scores[:, 0]
    return np.float32(loss_b.mean())
